# revision 16
# baseline (speedup 1.0000x reference)
"""Two-layer GAT on 8 Trainium2 NeuronCores (Bass/Tile SPMD kernel), v2.

Full inputs in, full output out. Structure:
  - host: bin-pack nodes into (core, tile, row) slots, build per-core edge
    metadata (int16 gather indices wrapped for dma_gather, per-tile local
    dst rows), fold attention vectors into augmented weights, cast to bf16.
  - device (SPMD, 8 cores):
    Phase A (replicated): every core computes the FULL table1
      rows [h1 (256) | al_s (4)] bf16 from a replicated xT — no collective.
    Phase B: per local dst-tile: dma_gather source rows (768B each),
      one-hot segment-softmax (exp without max subtraction; logits
      bounded), PSUM segment sums, ELU(+1) fused, layer-2 table rows
      [h2 (64) | al_s2 (1)] bf16 written to tbl2_shard.
      al_d per tile comes from a tiny matmul on the own-shard xTo input.
      AllGather of tbl2 is split into 4 row-chunks interleaved with the
      tile loop so the collective overlaps Phase B compute.
    Phase D: same edge machinery on tbl2 (256B rows) -> output rows f32.
"""

import heapq
import numpy as np
import ml_dtypes

import concourse.bacc as bacc
import concourse.bass as bass
import concourse.mybir as mybir
import concourse.tile as tile
from concourse.bass_utils import run_bass_kernel_spmd

dt = mybir.dt
f32 = dt.float32
bf16 = dt.bfloat16
npbf16 = ml_dtypes.bfloat16
NEG_SLOPE = 0.2
CLAMP = 60.0


class Cfg:
    def __init__(self, n=50000, f_in=128, heads=4, hid=64, out_ch=64,
                 ncores=8, nt=49, loch=12, hich=7, split=32768,
                 ag2_chunked=True):
        self.n = n
        self.f_in = f_in
        self.heads = heads
        self.hid = hid
        self.out_ch = out_ch
        self.ncores = ncores
        self.nt = nt                  # dst tiles per core
        self.ntr = nt * 128           # rows per core
        self.slots = ncores * self.ntr
        self.loch = loch              # lo-src gather chunks per tile
        self.hich = hich
        self.cpt = loch + hich
        self.split = split
        self.t1w = 384                # table1 row width (bf16) = 768B
        self.t2w = 128                # table2 row width (bf16) = 256B
        self.v1 = heads * hid         # 256 value cols (h), als at 256:260
        self.v2 = out_ch + 1          # 65: [h2 | als2]
        # AG2 tile-chunk boundaries (local tiles)
        self.ag2_chunked = ag2_chunked
        self.ag2_bounds = (0, 12, 24, 36, nt) if ag2_chunked else (0, nt)
        assert n <= self.slots
        assert self.split <= 32768
        assert self.slots - self.split <= 32768
        assert self.v1 + heads <= self.t1w
        assert self.v2 + 1 <= self.t2w


FULL = Cfg()


# ---------------------------------------------------------------------------
# Host-side preprocessing
# ---------------------------------------------------------------------------

def pack_nodes(cfg, dst_nodes):
    """Assign each node to a (tile, row) slot, balancing edge counts."""
    n, ntile = cfg.n, cfg.ncores * cfg.nt
    deg = np.bincount(dst_nodes, minlength=n)
    order = np.argsort(-deg, kind="stable")
    cap_edges = cfg.cpt * 128
    heap = [(0, t) for t in range(ntile)]
    heapq.heapify(heap)
    rows_used = np.zeros(ntile, np.int32)
    load = np.zeros(ntile, np.int64)
    tile_of = np.empty(n, np.int32)
    row_of = np.empty(n, np.int32)
    spill = []
    for nd in order:
        d = int(deg[nd])
        while True:
            l, t = heapq.heappop(heap)
            if rows_used[t] >= 128:
                continue
            if l + d > cap_edges and l > 0:
                spill.append((l, t))
                continue
            break
        for item in spill:
            heapq.heappush(heap, item)
        spill.clear()
        tile_of[nd] = t
        row_of[nd] = rows_used[t]
        rows_used[t] += 1
        load[t] += d
        heapq.heappush(heap, (l + d, t))
    perm = tile_of.astype(np.int64) * 128 + row_of
    return perm


def wrap16(a):
    """[nt, slots] int16 -> [128, nt*slots/16] wrapped in 16 partitions,
    replicated to 128."""
    ntl, s = a.shape
    w = a.reshape(ntl, s // 16, 16).transpose(0, 2, 1)   # [nt,16,s/16]
    w = w.transpose(1, 0, 2).reshape(16, ntl * (s // 16))
    return np.tile(w, (8, 1)).copy()


def prep_host(cfg, x, edge_index, W1, a_src1, a_dst1, b1, W2, a_src2, a_dst2, b2):
    n = cfg.n
    heads, hid, out_ch = cfg.heads, cfg.hid, cfg.out_ch
    x = np.asarray(x, np.float32)
    ei = np.asarray(edge_index, np.int64)
    loops = np.arange(n, dtype=np.int64)
    src = np.concatenate([ei[0], loops])
    dst = np.concatenate([ei[1], loops])

    perm = pack_nodes(cfg, dst)
    s_src = perm[src]
    s_dst = perm[dst]
    tile_g = s_dst // 128
    r_dst = s_dst % 128
    is_lo = s_src < cfg.split

    nt_all = cfg.ncores * cfg.nt
    lo_slots = cfg.loch * 128
    hi_slots = cfg.hich * 128

    idx_lo = np.zeros((nt_all, lo_slots), np.int16)
    idx_hi = np.zeros((nt_all, hi_slots), np.int16)
    dst_loc = np.full((nt_all, cfg.cpt * 128), -1.0, np.float32)

    key = tile_g * 2 + (~is_lo).astype(np.int64)
    order = np.argsort(key, kind="stable")
    ks = key[order]
    bounds = np.searchsorted(ks, np.arange(2 * nt_all + 1))
    for t in range(nt_all):
        elo = order[bounds[2 * t]:bounds[2 * t + 1]]
        ehi = order[bounds[2 * t + 1]:bounds[2 * t + 2]]
        nlo, nhi = len(elo), len(ehi)
        if nlo > lo_slots or nhi > hi_slots:
            raise RuntimeError(f"tile {t} overflow: lo={nlo} hi={nhi}")
        idx_lo[t, :nlo] = s_src[elo].astype(np.int16)
        idx_hi[t, :nhi] = (s_src[ehi] - cfg.split).astype(np.int16)
        dst_loc[t, :nlo] = r_dst[elo]
        dst_loc[t, lo_slots:lo_slots + nhi] = r_dst[ehi]

    # ---- folded weights ----
    W1 = np.asarray(W1, np.float32)
    W2 = np.asarray(W2, np.float32)
    a_src1 = np.asarray(a_src1, np.float32)
    a_dst1 = np.asarray(a_dst1, np.float32)
    a_src2 = np.asarray(a_src2, np.float32)
    a_dst2 = np.asarray(a_dst2, np.float32)
    Asrc = np.zeros((heads * hid, heads), np.float32)
    Adst = np.zeros((heads * hid, heads), np.float32)
    for h in range(heads):
        Asrc[h * hid:(h + 1) * hid, h] = a_src1[h]
        Adst[h * hid:(h + 1) * hid, h] = a_dst1[h]
    # w1aug: [128, 264] = [W1 (256) | W1@Asrc (4) | W1@Adst (4)]
    w1aug = np.concatenate([W1, W1 @ Asrc, W1 @ Adst], axis=1)

    # w2aug: [256, 66] = [W2 | W2@a_src2 | W2@a_dst2]; caug = -colsum(W2aug)
    w2aug = np.concatenate([W2, (W2 @ a_src2[0])[:, None],
                            (W2 @ a_dst2[0])[:, None]], axis=1)
    caug = -w2aug.sum(axis=0, keepdims=True)

    assert not np.any(np.asarray(b1)), "nonzero b1 unsupported"
    assert not np.any(np.asarray(b2)), "nonzero b2 unsupported"

    i128 = np.eye(128, dtype=npbf16)
    iotarow = np.tile(np.arange(128, dtype=npbf16), (128, 1)).copy()

    # xT: full, slot order, replicated
    xs = np.zeros((cfg.slots, cfg.f_in), np.float32)
    xs[perm] = x
    xT_full = np.ascontiguousarray(xs.T).astype(npbf16)

    idx_lo_w = wrap16(idx_lo)
    idx_hi_w = wrap16(idx_hi)
    lo_cols = lo_slots // 16
    hi_cols = hi_slots // 16

    npc = cfg.nt
    in_maps = []
    for c in range(cfg.ncores):
        t0, t1 = c * npc, (c + 1) * npc
        m = {
            "xT": xT_full,
            "xTo": np.ascontiguousarray(
                xT_full[:, c * cfg.ntr:(c + 1) * cfg.ntr]),
            "w1a": w1aug.astype(npbf16),
            "w2a": np.stack([w2aug[:128], w2aug[128:]]).astype(npbf16),
            "ca": caug.astype(npbf16),
            "i128": i128,
            "iota": iotarow,
            "idxlo": np.ascontiguousarray(
                idx_lo_w[:, t0 * lo_cols:t1 * lo_cols]),
            "idxhi": np.ascontiguousarray(
                idx_hi_w[:, t0 * hi_cols:t1 * hi_cols]),
            "dstloc": np.ascontiguousarray(
                dst_loc[t0:t1].reshape(npc, cfg.cpt, 128)
                .transpose(2, 0, 1).reshape(128, npc * cfg.cpt)),
        }
        in_maps.append(m)
    return in_maps, perm


# ---------------------------------------------------------------------------
# Device program
# ---------------------------------------------------------------------------

def build_program(cfg):
    H, HID = cfg.heads, cfg.hid
    V1, V2 = cfg.v1, cfg.v2          # 256, 65
    T1W, T2W = cfg.t1w, cfg.t2w      # 384, 128
    NT, CPT, LOCH, HICH = cfg.nt, cfg.cpt, cfg.loch, cfg.hich
    NTR = cfg.ntr
    OUT = cfg.out_ch
    SPLIT = cfg.split
    K2 = H * HID                     # 256
    n_k2 = K2 // 128                 # 2
    NST = cfg.slots // 1024          # 49 supertiles in phase A
    W1C = V1 + 2 * H                 # 264

    nc = bacc.Bacc("TRN2", target_bir_lowering=False, debug=False,
                   num_devices=cfg.ncores)

    xT = nc.dram_tensor("xT", [cfg.f_in, cfg.slots], bf16, kind="ExternalInput")
    xTo_d = nc.dram_tensor("xTo", [cfg.f_in, NTR], bf16, kind="ExternalInput")
    w1a_d = nc.dram_tensor("w1a", [cfg.f_in, W1C], bf16, kind="ExternalInput")
    w2a_d = nc.dram_tensor("w2a", [n_k2, 128, V2 + 1], bf16, kind="ExternalInput")
    ca_d = nc.dram_tensor("ca", [1, V2 + 1], bf16, kind="ExternalInput")
    i128_d = nc.dram_tensor("i128", [128, 128], bf16, kind="ExternalInput")
    iota_d = nc.dram_tensor("iota", [128, 128], bf16, kind="ExternalInput")
    idxlo_d = nc.dram_tensor("idxlo", [128, NT * LOCH * 8], dt.int16, kind="ExternalInput")
    idxhi_d = nc.dram_tensor("idxhi", [128, NT * HICH * 8], dt.int16, kind="ExternalInput")
    dstloc_d = nc.dram_tensor("dstloc", [128, NT * CPT], f32, kind="ExternalInput")
    out_d = nc.dram_tensor("out_shard", [NTR, OUT], f32, kind="ExternalOutput")

    tbl1 = nc.dram_tensor("tbl1", [cfg.slots, T1W], bf16)
    tbl2_shard = nc.dram_tensor("tbl2_shard", [NTR, T2W], bf16)
    tbl2 = nc.dram_tensor("tbl2", [cfg.slots, T2W], bf16, addr_space="Shared")

    rg = [list(range(cfg.ncores))]
    AG2B = cfg.ag2_bounds

    with tile.TileContext(nc) as tc:
        with tc.tile_pool(name="res", bufs=1) as res:
            w1a = res.tile([cfg.f_in, W1C], bf16)
            w2a = res.tile([128, n_k2 * (V2 + 1)], bf16)
            ca = res.tile([1, V2 + 1], bf16)
            i128 = res.tile([128, 128], bf16)
            iota = res.tile([128, 128], bf16)
            idxlo = res.tile([128, NT * LOCH * 8], dt.int16)
            idxhi = res.tile([128, NT * HICH * 8], dt.int16)
            dstloc = res.tile([128, NT * CPT], f32)
            xTo = res.tile([cfg.f_in, NTR], bf16)
            alds2 = res.tile([128, NT], bf16)
            ones_row = res.tile([1, 128], bf16)

            nc.sync.dma_start(w1a[:], w1a_d[:])
            for j in range(n_k2):
                nc.sync.dma_start(w2a[:, j * (V2 + 1):(j + 1) * (V2 + 1)],
                                  w2a_d[j, :, :])
            nc.sync.dma_start(ca[:], ca_d[:])
            nc.sync.dma_start(i128[:], i128_d[:])
            nc.sync.dma_start(iota[:], iota_d[:])
            nc.sync.dma_start(idxlo[:], idxlo_d[:])
            nc.sync.dma_start(idxhi[:], idxhi_d[:])
            nc.sync.dma_start(dstloc[:], dstloc_d[:])
            nc.sync.dma_start(xTo[:], xTo_d[:])
            nc.gpsimd.memset(ones_row[:], 1.0)

            # ------------------------------------------------------------
            # Phase A (replicated): full table1 = [h1 | als], bf16
            # ------------------------------------------------------------
            with tc.tile_pool(name="pa_sb", bufs=2) as pa, \
                 tc.tile_pool(name="pa_ps", bufs=2, space="PSUM") as pap:
                for st in range(NST):
                    xg = pa.tile([128, 1024], bf16, tag="xg")
                    nc.sync.dma_start(xg[:], xT[:, st * 1024:(st + 1) * 1024])
                    stg = pa.tile([128, 8 * 260], bf16, tag="stg")
                    for sub in range(8):
                        ps = pap.tile([128, W1C], f32, tag="h1")
                        nc.tensor.matmul(ps[:], xg[:, sub * 128:(sub + 1) * 128],
                                         w1a[:], start=True, stop=True)
                        nc.any.tensor_copy(stg[:, sub * 260:(sub + 1) * 260],
                                           ps[:, 0:260])
                    dst_ap = tbl1[st * 1024:(st + 1) * 1024, 0:260].rearrange(
                        "(s p) c -> p s c", p=128)
                    nc.sync.dma_start(
                        dst_ap, stg[:].rearrange("p (s c) -> p s c", c=260))

            # ------------------------------------------------------------
            # Edge phases
            # ------------------------------------------------------------
            def edge_phase(lay, tblw, tbl_full, nvals, nheads, evict_fn,
                           ag2_emit=None):
                # gathered row: [values (nvals) | als (nheads) | pad]
                # scl/seg row:  [values*w (nvals) | w (nheads)]
                segw = nvals + nheads
                with tc.tile_pool(name=f"eb{lay}", bufs=2) as eb, \
                     tc.tile_pool(name=f"oh{lay}", bufs=2) as ohp, \
                     tc.tile_pool(name=f"oht{lay}", bufs=2) as ohtp, \
                     tc.tile_pool(name=f"sc{lay}", bufs=3) as scp, \
                     tc.tile_pool(name=f"ev{lay}", bufs=2) as ev, \
                     tc.tile_pool(name=f"ps{lay}", bufs=1, space="PSUM") as ps:
                    for t in range(NT):
                        if ag2_emit is not None:
                            ag2_emit(t)
                        gbuf = eb.tile([128, CPT * tblw], bf16, tag="gbuf")
                        g3 = gbuf[:].rearrange("p (c w) -> p c w", w=tblw)
                        lo_i = idxlo[:, t * LOCH * 8:(t + 1) * LOCH * 8]
                        hi_i = idxhi[:, t * HICH * 8:(t + 1) * HICH * 8]
                        nc.gpsimd.dma_gather(
                            g3[:, 0:LOCH, :], tbl_full[0:SPLIT, :],
                            lo_i, LOCH * 128, LOCH * 128, tblw,
                            single_packet=False)
                        nc.gpsimd.dma_gather(
                            g3[:, LOCH:CPT, :], tbl_full[SPLIT:cfg.slots, :],
                            hi_i, HICH * 128, HICH * 128, tblw,
                            single_packet=False)

                        # one-hots [e, d] and transposed [d, e]
                        oh_all = ohp.tile([128, CPT * 128], bf16, tag="oh")
                        for cc in range(CPT):
                            nc.vector.tensor_scalar(
                                oh_all[:, cc * 128:(cc + 1) * 128], iota[:],
                                dstloc[:, t * CPT + cc:t * CPT + cc + 1], None,
                                mybir.AluOpType.is_equal)
                        ohT_all = ohtp.tile([128, CPT * 128], bf16, tag="ohT")
                        for cc in range(CPT):
                            ohT_ps = ps.tile([128, 128], bf16, tag="ohT", bufs=2)
                            nc.tensor.transpose(
                                ohT_ps[:], oh_all[:, cc * 128:(cc + 1) * 128],
                                i128[:])
                            nc.any.tensor_copy(
                                ohT_all[:, cc * 128:(cc + 1) * 128], ohT_ps[:])

                        # al_d for this tile
                        if lay == 1:
                            ald_ps = ps.tile([128, H], f32, tag="ald", bufs=1)
                            nc.tensor.matmul(
                                ald_ps[:], xTo[:, t * 128:(t + 1) * 128],
                                w1a[:, V1 + H:V1 + 2 * H], start=True, stop=True)
                            ald = ev.tile([128, H], bf16, tag="ald_sb")
                            nc.any.tensor_copy(ald[:], ald_ps[:])
                            ald_ap = ald[:]
                        else:
                            ald_ap = alds2[:, t:t + 1]

                        # epre[e, cc, h] = als[src] + ald[dst]
                        epre_ps = ps.tile([128, CPT * nheads], f32, tag="epre",
                                          bufs=1)
                        ep3 = epre_ps[:].rearrange("p (c h) -> p c h", h=nheads)
                        als_view = g3[:, :, nvals:nvals + nheads]
                        nc.tensor.matmul(ep3, i128[:], als_view,
                                         start=True, stop=False,
                                         skip_group_check=True)
                        for cc in range(CPT):
                            nc.tensor.matmul(
                                ep3[:, cc, :],
                                ohT_all[:, cc * 128:(cc + 1) * 128], ald_ap,
                                start=False, stop=(cc == CPT - 1),
                                skip_group_check=True)

                        # w = exp(clamp(lrelu(epre)))
                        nh = CPT * nheads
                        elr = ev.tile([128, nh], f32, tag="elr")
                        nc.vector.tensor_scalar(
                            elr[:], epre_ps[:], NEG_SLOPE, None,
                            mybir.AluOpType.mult)
                        nc.vector.tensor_tensor(elr[:], elr[:], epre_ps[:],
                                                mybir.AluOpType.max)
                        nc.vector.tensor_scalar(elr[:], elr[:], CLAMP, None,
                                                mybir.AluOpType.min)
                        wexp = ev.tile([128, nh], bf16, tag="wexp")
                        nc.scalar.activation(wexp[:], elr[:],
                                             mybir.ActivationFunctionType.Exp)
                        w3 = wexp[:].rearrange("p (c h) -> p c h", h=nheads)

                        # scale values + segment-sum via one-hot matmuls
                        seg_ps = ps.tile([128, segw], f32, tag="seg", bufs=2)
                        for cc in range(CPT):
                            scl = scp.tile([128, segw], bf16, tag="scl")
                            s3 = scl[:, 0:nvals].rearrange(
                                "p (h u) -> p h u", h=nheads)
                            gv = g3[:, cc, 0:nvals].rearrange(
                                "p (h u) -> p h u", h=nheads)
                            wv = w3[:, cc, :].unsqueeze(2).broadcast_to(
                                [128, nheads, nvals // nheads])
                            nc.vector.tensor_tensor(s3, gv, wv,
                                                    mybir.AluOpType.mult)
                            nc.vector.tensor_copy(scl[:, nvals:nvals + nheads],
                                                  w3[:, cc, :])
                            nc.tensor.matmul(seg_ps[:],
                                             oh_all[:, cc * 128:(cc + 1) * 128],
                                             scl[:],
                                             start=(cc == 0), stop=(cc == CPT - 1))
                        evict_fn(t, seg_ps, (eb, ev, ps))

            # ---- layer-1 eviction: ELU+1 -> table2 rows ----
            def evict1(t, seg_ps, pools):
                eb, ev, ps = pools
                den = seg_ps[:, V1:V1 + H]                     # [128, H]
                denf = ev.tile([128, H], f32, tag="denf")
                nc.vector.tensor_scalar(denf[:], den, 0.0, None,
                                        mybir.AluOpType.is_equal)
                nc.vector.tensor_tensor(denf[:], denf[:], den,
                                        mybir.AluOpType.add)
                rec = ev.tile([128, H], f32, tag="rec")
                nc.vector.reciprocal(rec[:], denf[:])
                pe = ev.tile([128, K2], f32, tag="pelu")
                p3 = pe[:].rearrange("p (h u) -> p h u", h=H)
                sg3 = seg_ps[:, 0:V1].rearrange("p (h u) -> p h u", h=H)
                r3 = rec[:].unsqueeze(2).broadcast_to([128, H, HID])
                nc.vector.tensor_tensor(p3, sg3, r3, mybir.AluOpType.mult)
                # elu(v) = max(v,0) + (exp(min(v,0)) - 1); -1 kept explicit
                # (folding it into the weights amplifies bf16 rounding via
                # cancellation of the +1-shifted intermediate)
                mn = ev.tile([128, K2], f32, tag="mn")
                nc.vector.tensor_scalar(mn[:], pe[:], 0.0, None,
                                        mybir.AluOpType.min)
                nc.scalar.activation(mn[:], mn[:],
                                     mybir.ActivationFunctionType.Exp)
                nc.vector.tensor_scalar(mn[:], mn[:], -1.0, None,
                                        mybir.AluOpType.add)
                nc.vector.tensor_scalar(pe[:], pe[:], 0.0, None,
                                        mybir.AluOpType.max)
                pre = ev.tile([128, K2], bf16, tag="pre")
                nc.vector.tensor_tensor(pre[:], pe[:], mn[:],
                                        mybir.AluOpType.add)
                # h2 rows: [h2 | als2 | ald2] = elu @ W2aug
                h2_ps = ps.tile([128, V2 + 1], f32, tag="h2", bufs=1)
                for j in range(n_k2):
                    peT_ps = ps.tile([128, 128], bf16, tag="peT", bufs=1)
                    nc.tensor.transpose(peT_ps[:], pre[:, j * 128:(j + 1) * 128],
                                        i128[:])
                    peT = ev.tile([128, 128], bf16, tag="peT_sb")
                    nc.any.tensor_copy(peT[:], peT_ps[:])
                    nc.tensor.matmul(h2_ps[:], peT[:],
                                     w2a[:, j * (V2 + 1):(j + 1) * (V2 + 1)],
                                     start=(j == 0), stop=(j == n_k2 - 1),
                                     skip_group_check=True)
                stg2 = eb.tile([128, T2W], bf16, tag="stg2")
                nc.any.tensor_copy(stg2[:, 0:V2], h2_ps[:, 0:V2])
                nc.any.tensor_copy(alds2[:, t:t + 1], h2_ps[:, V2:V2 + 1])
                nc.sync.dma_start(tbl2_shard[t * 128:(t + 1) * 128, :], stg2[:])

            # chunked AllGather of tbl2, interleaved with phase B tiles.
            # Collective outputs must be contiguous, so each chunk gathers
            # into its own tensor, then a local DRAM->DRAM dma splices it
            # into the unified slot-ordered tbl2 (both overlap phase B).
            tbl2_3d = tbl2[:].rearrange("(c r) w -> c r w", c=cfg.ncores)
            tbl2_chunks = []
            if cfg.ag2_chunked:
                for k in range(len(AG2B) - 1):
                    rows_k = (AG2B[k + 1] - AG2B[k]) * 128
                    tbl2_chunks.append(nc.dram_tensor(
                        f"tbl2c{k}", [cfg.ncores * rows_k, T2W], bf16,
                        addr_space="Shared"))

            def ag2_chunk(k):
                r0, r1 = AG2B[k] * 128, AG2B[k + 1] * 128
                nc.gpsimd.collective_compute(
                    "AllGather", mybir.AluOpType.bypass, replica_groups=rg,
                    ins=[tbl2_shard[r0:r1, :]], outs=[tbl2_chunks[k][:]])
                nc.sync.dma_start(
                    tbl2_3d[:, r0:r1, :],
                    tbl2_chunks[k][:].rearrange("(c r) w -> c r w",
                                                c=cfg.ncores))

            def ag2_emit(t):
                for k in range(len(AG2B) - 2):
                    if t == AG2B[k + 1] + 2:
                        ag2_chunk(k)

            edge_phase(1, T1W, tbl1, V1, H, evict1, ag2_emit=ag2_emit)

            if cfg.ag2_chunked:
                ag2_chunk(len(AG2B) - 2)
            else:
                nc.gpsimd.collective_compute(
                    "AllGather", mybir.AluOpType.bypass, replica_groups=rg,
                    ins=[tbl2_shard[:]], outs=[tbl2[:]])

            # ---- layer-2 eviction: output rows ----
            def evict2(t, seg_ps, pools):
                eb, ev, ps = pools
                den = seg_ps[:, OUT:OUT + 1]
                denf = ev.tile([128, 1], f32, tag="denf2")
                nc.vector.tensor_scalar(denf[:], den, 0.0, None,
                                        mybir.AluOpType.is_equal)
                nc.vector.tensor_tensor(denf[:], denf[:], den,
                                        mybir.AluOpType.add)
                rec = ev.tile([128, 1], f32, tag="rec2")
                nc.vector.reciprocal(rec[:], denf[:])
                ot = ev.tile([128, OUT], f32, tag="ot")
                nc.vector.tensor_scalar(ot[:], seg_ps[:, 0:OUT], rec[:], None,
                                        mybir.AluOpType.mult)
                nc.sync.dma_start(out_d[t * 128:(t + 1) * 128, :], ot[:])

            edge_phase(2, T2W, tbl2, OUT, 1, evict2)

    nc.compile()
    return nc


# ---------------------------------------------------------------------------
# Entry point
# ---------------------------------------------------------------------------

_CACHE = {}


def _get_program(cfg):
    key = tuple(sorted((k, v) for k, v in cfg.__dict__.items()))
    if key not in _CACHE:
        _CACHE[key] = build_program(cfg)
    return _CACHE[key]


def run(cfg, inputs, trace=False, **kw):
    in_maps, perm = prep_host(cfg, **inputs)
    nc = _get_program(cfg)
    res = run_bass_kernel_spmd(nc, in_maps, list(range(cfg.ncores)),
                               trace=trace, **kw)
    out_full = np.concatenate([res.results[c]["out_shard"]
                               for c in range(cfg.ncores)], axis=0)
    return out_full[perm[:cfg.n]].astype(np.float32), res


def kernel(x, edge_index, W1, a_src1, a_dst1, b1, W2, a_src2, a_dst2, b2):
    out, _ = run(FULL, dict(x=x, edge_index=edge_index, W1=W1,
                            a_src1=a_src1, a_dst1=a_dst1, b1=b1,
                            W2=W2, a_src2=a_src2, a_dst2=a_dst2, b2=b2))
    return out
